# revision 1
# baseline (speedup 1.0000x reference)
"""Trainium2 kernel for nn_Gtu2d (Gated Toeplitz Unit 2D).

Strategy (8 NeuronCores, single chip):
  - Data-parallel over batch (8 batches -> 8 cores) for the projections,
    FFT mixing, gating and output projection.
  - The RPE coefficient MLP (position grid 111x111) is sharded by grid
    rows across the 8 cores (14 rows each, padded to 112); the resulting
    Toeplitz coefficients g are exchanged with an all_gather so every
    core holds the full coefficient tensor for its batch's FFT mixing.
  - The 2D rFFT2 circulant convolution is expressed as dense DFT
    matmuls (cos/sin matrices, explicit real/imag arithmetic) so it all
    runs on the TensorEngine; the circulant embedding of the 111x111
    coefficient block into the 112x112 grid is folded into the DFT
    matrices as a phase shift (no scatter needed).

Everything runs in one pmap over the 8 cores via PJRT.
"""

import numpy as np
import jax
import jax.numpy as jnp
from functools import partial

EMBED_DIM = 512
NUM_HEADS = 8
D1 = 1024
HEAD_DIM = 128
RPE_DIM = 512
RPE_LAYERS = 3
N = 56
M = 56
BATCH = 8
EPS = 1e-8
P = 2 * N          # 112 padded FFT size (height)
Q = 2 * M          # 112 padded FFT size (width)
LW = Q // 2 + 1    # 57 rfft width bins
NCORES = 8
ROWS_PER_CORE = 14  # 8*14 = 112 >= 111 grid rows (last row masked out)

# ---------------------------------------------------------------- DFT mats
def _build_mats():
    n = np.arange(N)
    k = np.arange(P)
    l = np.arange(LW)
    # forward height DFT (56 -> 112 full complex), e^{-2pi i n k / P}
    ang_h = -2.0 * np.pi * np.outer(n, k) / P
    EHc = np.cos(ang_h).astype(np.float32)          # (56,112)
    EHs = np.sin(ang_h).astype(np.float32)
    # forward width rDFT (56 -> 57)
    ang_w = -2.0 * np.pi * np.outer(n, l) / Q
    EWc = np.cos(ang_w).astype(np.float32)          # (56,57)
    EWs = np.sin(ang_w).astype(np.float32)
    # coefficient DFTs with phase shift for circulant embedding:
    # row p of g corresponds to offset (p-55); position in circulant grid is
    # (p-55) mod 112, and e^{-2pi i k ((p-55) mod 112)/112} = e^{-2pi i k (p-55)/112}
    p_idx = np.arange(112)  # padded to 112 rows (row 111 is garbage, masked)
    ang_gh = -2.0 * np.pi * np.outer(p_idx - 55, k) / P
    GHc = np.cos(ang_gh).astype(np.float32)         # (112,112)
    GHs = np.sin(ang_gh).astype(np.float32)
    GHc[111] = 0.0  # mask padded garbage row
    GHs[111] = 0.0
    q_idx = np.arange(111)
    ang_gw = -2.0 * np.pi * np.outer(q_idx - 55, l) / Q
    GWc = np.cos(ang_gw).astype(np.float32)         # (111,57)
    GWs = np.sin(ang_gw).astype(np.float32)
    # inverse: y[n,m] = (1/P/Q) Re( sum_k sum_l w_l F[k,l] e^{2pi i(kn+lm)/112} )
    wl = np.ones(LW, np.float32) * 2.0
    wl[0] = 1.0
    wl[LW - 1] = 1.0
    ang_ih = 2.0 * np.pi * np.outer(k, n) / P
    IHc = np.cos(ang_ih).astype(np.float32)         # (112,56)
    IHs = np.sin(ang_ih).astype(np.float32)
    ang_iw = 2.0 * np.pi * np.outer(l, n) / Q
    IWc = (wl[:, None] * np.cos(ang_iw) / (P * Q)).astype(np.float32)  # (57,56)
    IWs = (wl[:, None] * np.sin(ang_iw) / (P * Q)).astype(np.float32)
    return EHc, EHs, EWc, EWs, GHc, GHs, GWc, GWs, IHc, IHs, IWc, IWs

_MATS = _build_mats()

# static relative-coordinate grid, padded to 112 rows
_DP = np.arange(-(N - 1), N + 1, dtype=np.float32)      # 112 values (last pad)
_DQ = np.arange(-(M - 1), M, dtype=np.float32)          # 111
_COORDS = np.stack(np.meshgrid(_DP, _DQ, indexing='ij'), axis=-1)  # (112,111,2)


def _rms(x):
    nrm = jnp.sqrt(jnp.sum(x * x, axis=-1, keepdims=True))
    return x / (nrm * (x.shape[-1] ** -0.5) + EPS)


def _core_fn(x_b, coords_sh, W_u, b_u, W_v, b_v, W_o, b_o,
             rpe_in_w, rpe_in_b, rpe_h_w, rpe_h_b, rpe_out_w, rpe_out_b):
    (EHc, EHs, EWc, EWs, GHc, GHs, GWc, GWs, IHc, IHs, IWc, IWs) = [
        jnp.asarray(m) for m in _MATS]

    # ---- RPE MLP on this core's shard of grid rows -> g shard
    t = coords_sh.reshape(-1, 2) @ rpe_in_w.T + rpe_in_b      # (14*111, 512)
    for i in range(RPE_LAYERS):
        t = jax.nn.relu(_rms(t)) @ rpe_h_w[i].T + rpe_h_b[i]
    g_sh = jax.nn.relu(_rms(t)) @ rpe_out_w.T + rpe_out_b     # (14*111, 1024)

    # ---- exchange: full coefficient tensor on every core
    g = jax.lax.all_gather(g_sh, 'i')                          # (8, 14*111, 1024)
    g = g.reshape(112, 111, D1)                                # row 111 = garbage (masked in GH)

    # ---- coefficient spectrum Af (112, 57, D1) complex
    # contract p (112 padded rows) then q (111 cols)
    t1r = jnp.einsum('pqc,pk->kqc', g, GHc)
    t1i = jnp.einsum('pqc,pk->kqc', g, GHs)
    Afr = jnp.einsum('kqc,ql->klc', t1r, GWc) - jnp.einsum('kqc,ql->klc', t1i, GWs)
    Afi = jnp.einsum('kqc,ql->klc', t1r, GWs) + jnp.einsum('kqc,ql->klc', t1i, GWc)

    # ---- main path for this core's batch element
    x = x_b                                                    # (56,56,512)
    xn = _rms(x)
    u = jax.nn.silu(xn @ W_u.T + b_u)                          # (56,56,1024)
    v = jax.nn.silu(xn @ W_v.T + b_v)                          # (56,56,1024)

    # forward 2D DFT of v (zero padding folded: only 56 rows/cols exist)
    s1r = jnp.einsum('nmc,nk->kmc', v, EHc)                    # (112,56,D1)
    s1i = jnp.einsum('nmc,nk->kmc', v, EHs)
    Vfr = jnp.einsum('kmc,ml->klc', s1r, EWc) - jnp.einsum('kmc,ml->klc', s1i, EWs)
    Vfi = jnp.einsum('kmc,ml->klc', s1r, EWs) + jnp.einsum('kmc,ml->klc', s1i, EWc)

    # pointwise complex product
    Pr = Vfr * Afr - Vfi * Afi
    Pi = Vfr * Afi + Vfi * Afr

    # inverse: contract k then l, keep real part
    z1r = jnp.einsum('klc,kn->nlc', Pr, IHc) - jnp.einsum('klc,kn->nlc', Pi, IHs)
    z1i = jnp.einsum('klc,kn->nlc', Pr, IHs) + jnp.einsum('klc,kn->nlc', Pi, IHc)
    y = jnp.einsum('nlc,lm->nmc', z1r, IWc) - jnp.einsum('nlc,lm->nmc', z1i, IWs)

    out = (u * y) @ W_o.T + b_o + x                            # (56,56,512)
    return out


_PMAPPED = None


def _get_pmapped():
    global _PMAPPED
    if _PMAPPED is None:
        _PMAPPED = jax.pmap(
            _core_fn, axis_name='i',
            in_axes=(0, 0) + (None,) * 12,
            devices=jax.devices()[:NCORES])
    return _PMAPPED


def kernel(x, W_u, b_u, W_v, b_v, W_o, b_o,
           rpe_in_w, rpe_in_b, rpe_h_w, rpe_h_b, rpe_out_w, rpe_out_b,
           H, W):
    x = np.asarray(x, np.float32)
    coords = _COORDS.reshape(NCORES, ROWS_PER_CORE * 111, 2)
    f = _get_pmapped()
    out = f(x, jnp.asarray(coords),
            jnp.asarray(W_u), jnp.asarray(b_u),
            jnp.asarray(W_v), jnp.asarray(b_v),
            jnp.asarray(W_o), jnp.asarray(b_o),
            jnp.asarray(rpe_in_w), jnp.asarray(rpe_in_b),
            jnp.asarray(rpe_h_w), jnp.asarray(rpe_h_b),
            jnp.asarray(rpe_out_w), jnp.asarray(rpe_out_b))
    return np.asarray(out, np.float32)


# revision 3
# speedup vs baseline: 22.9004x; 22.9004x over previous
"""Trainium2 kernel for nn_Gtu2d (Gated Toeplitz Unit 2D).

Strategy (8 NeuronCores, single chip):
  - Data-parallel over batch (8 batches -> 8 cores) for the projections,
    FFT mixing, gating and output projection.
  - The RPE coefficient MLP (position grid 111x111) is sharded by grid
    rows across the 8 cores (14 rows each, padded to 112); the resulting
    Toeplitz coefficients g are exchanged with an all_gather so every
    core holds the full coefficient tensor for its batch's FFT mixing.
  - The 2D rFFT2 circulant convolution is expressed as dense DFT
    matmuls (cos/sin matrices, explicit real/imag arithmetic) so it all
    runs on the TensorEngine; the circulant embedding of the 111x111
    coefficient block into the 112x112 grid is folded into the DFT
    matrices as a phase shift (no scatter needed).

Everything runs in one pmap over the 8 cores via PJRT.
"""

import numpy as np
import jax
import jax.numpy as jnp
from functools import partial

EMBED_DIM = 512
NUM_HEADS = 8
D1 = 1024
HEAD_DIM = 128
RPE_DIM = 512
RPE_LAYERS = 3
N = 56
M = 56
BATCH = 8
EPS = 1e-8
P = 2 * N          # 112 padded FFT size (height)
Q = 2 * M          # 112 padded FFT size (width)
LW = Q // 2 + 1    # 57 rfft width bins
NCORES = 8
ROWS_PER_CORE = 14  # 8*14 = 112 >= 111 grid rows (last row masked out)

# ---------------------------------------------------------------- DFT mats
def _build_mats():
    n = np.arange(N)
    k = np.arange(P)
    l = np.arange(LW)
    # forward height DFT (56 -> 112 full complex), e^{-2pi i n k / P}
    ang_h = -2.0 * np.pi * np.outer(n, k) / P
    EHc = np.cos(ang_h).astype(np.float32)          # (56,112)
    EHs = np.sin(ang_h).astype(np.float32)
    # forward width rDFT (56 -> 57)
    ang_w = -2.0 * np.pi * np.outer(n, l) / Q
    EWc = np.cos(ang_w).astype(np.float32)          # (56,57)
    EWs = np.sin(ang_w).astype(np.float32)
    # coefficient DFTs with phase shift for circulant embedding:
    # row p of g corresponds to offset (p-55); position in circulant grid is
    # (p-55) mod 112, and e^{-2pi i k ((p-55) mod 112)/112} = e^{-2pi i k (p-55)/112}
    p_idx = np.arange(112)  # padded to 112 rows (row 111 is garbage, masked)
    ang_gh = -2.0 * np.pi * np.outer(p_idx - 55, k) / P
    GHc = np.cos(ang_gh).astype(np.float32)         # (112,112)
    GHs = np.sin(ang_gh).astype(np.float32)
    GHc[111] = 0.0  # mask padded garbage row
    GHs[111] = 0.0
    q_idx = np.arange(111)
    ang_gw = -2.0 * np.pi * np.outer(q_idx - 55, l) / Q
    GWc = np.cos(ang_gw).astype(np.float32)         # (111,57)
    GWs = np.sin(ang_gw).astype(np.float32)
    # inverse: y[n,m] = (1/P/Q) Re( sum_k sum_l w_l F[k,l] e^{2pi i(kn+lm)/112} )
    wl = np.ones(LW, np.float32) * 2.0
    wl[0] = 1.0
    wl[LW - 1] = 1.0
    ang_ih = 2.0 * np.pi * np.outer(k, n) / P
    IHc = np.cos(ang_ih).astype(np.float32)         # (112,56)
    IHs = np.sin(ang_ih).astype(np.float32)
    ang_iw = 2.0 * np.pi * np.outer(l, n) / Q
    IWc = (wl[:, None] * np.cos(ang_iw) / (P * Q)).astype(np.float32)  # (57,56)
    IWs = (wl[:, None] * np.sin(ang_iw) / (P * Q)).astype(np.float32)
    return EHc, EHs, EWc, EWs, GHc, GHs, GWc, GWs, IHc, IHs, IWc, IWs

_MATS = _build_mats()

# static relative-coordinate grid, padded to 112 rows
_DP = np.arange(-(N - 1), N + 1, dtype=np.float32)      # 112 values (last pad)
_DQ = np.arange(-(M - 1), M, dtype=np.float32)          # 111
_COORDS = np.stack(np.meshgrid(_DP, _DQ, indexing='ij'), axis=-1)  # (112,111,2)


def _rms(x):
    nrm = jnp.sqrt(jnp.sum(x * x, axis=-1, keepdims=True))
    return x / (nrm * (x.shape[-1] ** -0.5) + EPS)


def _core_fn(x_b, coords_sh, W_u, b_u, W_v, b_v, W_o, b_o,
             rpe_in_w, rpe_in_b, rpe_h_w, rpe_h_b, rpe_out_w, rpe_out_b):
    (EHc, EHs, EWc, EWs, GHc, GHs, GWc, GWs, IHc, IHs, IWc, IWs) = [
        jnp.asarray(m) for m in _MATS]

    # ---- RPE MLP on this core's shard of grid rows -> g shard
    t = coords_sh.reshape(-1, 2) @ rpe_in_w.T + rpe_in_b      # (14*111, 512)
    for i in range(RPE_LAYERS):
        t = jax.nn.relu(_rms(t)) @ rpe_h_w[i].T + rpe_h_b[i]
    g_sh = jax.nn.relu(_rms(t)) @ rpe_out_w.T + rpe_out_b     # (14*111, 1024)

    # ---- exchange: full coefficient tensor on every core, but only this
    # core's head slice of channels (head-parallel coefficient FFT)
    g = jax.lax.all_gather(g_sh, 'i')                          # (8, 14*111, 1024)
    g = g.reshape(112, 111, NUM_HEADS, HEAD_DIM)               # row 111 = garbage (masked in GH)
    h = jax.lax.axis_index('i')
    gh = jax.lax.dynamic_slice_in_dim(g, h, 1, axis=2)[:, :, 0]  # (112, 111, 128)

    # ---- head-slice coefficient spectrum (112, 57, 128) complex
    # contract q first (111 -> 57, shrinking), then p (111 -> 112)
    t1r = jnp.einsum('pqd,ql->pld', gh, GWc)
    t1i = jnp.einsum('pqd,ql->pld', gh, GWs)
    Ahr = jnp.einsum('pld,pk->kld', t1r, GHc) - jnp.einsum('pld,pk->kld', t1i, GHs)
    Ahi = jnp.einsum('pld,pk->kld', t1r, GHs) + jnp.einsum('pld,pk->kld', t1i, GHc)

    # gather the full spectrum: (8, 112, 57, 128) -> (112, 57, 1024)
    Afr = jnp.moveaxis(jax.lax.all_gather(Ahr, 'i'), 0, 2).reshape(P, LW, D1)
    Afi = jnp.moveaxis(jax.lax.all_gather(Ahi, 'i'), 0, 2).reshape(P, LW, D1)

    # ---- main path for this core's batch element
    x = x_b                                                    # (56,56,512)
    xn = _rms(x)
    u = jax.nn.silu(xn @ W_u.T + b_u)                          # (56,56,1024)
    v = jax.nn.silu(xn @ W_v.T + b_v)                          # (56,56,1024)

    # forward 2D DFT of v (zero padding folded: only 56 rows/cols exist)
    # width rfft first (56 -> 57, shrinking), then height (56 -> 112)
    s1r = jnp.einsum('nmc,ml->nlc', v, EWc)                    # (56,57,D1)
    s1i = jnp.einsum('nmc,ml->nlc', v, EWs)
    Vfr = jnp.einsum('nlc,nk->klc', s1r, EHc) - jnp.einsum('nlc,nk->klc', s1i, EHs)
    Vfi = jnp.einsum('nlc,nk->klc', s1r, EHs) + jnp.einsum('nlc,nk->klc', s1i, EHc)

    # pointwise complex product
    Pr = Vfr * Afr - Vfi * Afi
    Pi = Vfr * Afi + Vfi * Afr

    # inverse: contract k then l, keep real part
    z1r = jnp.einsum('klc,kn->nlc', Pr, IHc) - jnp.einsum('klc,kn->nlc', Pi, IHs)
    z1i = jnp.einsum('klc,kn->nlc', Pr, IHs) + jnp.einsum('klc,kn->nlc', Pi, IHc)
    y = jnp.einsum('nlc,lm->nmc', z1r, IWc) - jnp.einsum('nlc,lm->nmc', z1i, IWs)

    out = (u * y) @ W_o.T + b_o + x                            # (56,56,512)
    return out


_PMAPPED = None


def _get_pmapped():
    global _PMAPPED
    if _PMAPPED is None:
        _PMAPPED = jax.pmap(
            _core_fn, axis_name='i',
            in_axes=(0, 0) + (None,) * 12,
            devices=jax.devices()[:NCORES])
    return _PMAPPED


def kernel(x, W_u, b_u, W_v, b_v, W_o, b_o,
           rpe_in_w, rpe_in_b, rpe_h_w, rpe_h_b, rpe_out_w, rpe_out_b,
           H, W):
    x = np.asarray(x, np.float32)
    coords = _COORDS.reshape(NCORES, ROWS_PER_CORE * 111, 2)
    f = _get_pmapped()
    out = f(x, jnp.asarray(coords),
            jnp.asarray(W_u), jnp.asarray(b_u),
            jnp.asarray(W_v), jnp.asarray(b_v),
            jnp.asarray(W_o), jnp.asarray(b_o),
            jnp.asarray(rpe_in_w), jnp.asarray(rpe_in_b),
            jnp.asarray(rpe_h_w), jnp.asarray(rpe_h_b),
            jnp.asarray(rpe_out_w), jnp.asarray(rpe_out_b))
    return np.asarray(out, np.float32)


# revision 6
# speedup vs baseline: 22.9492x; 1.0021x over previous
"""Trainium2 kernel for nn_Gtu2d (Gated Toeplitz Unit 2D).

Strategy (8 NeuronCores, single chip):
  - Data-parallel over batch (8 batches -> 8 cores) for the projections,
    FFT mixing, gating and output projection.
  - The RPE coefficient MLP (position grid 111x111) is sharded by grid
    rows across the 8 cores (14 rows each, padded to 112); the resulting
    Toeplitz coefficients g are exchanged with an all_gather so every
    core holds the full coefficient tensor for its batch's FFT mixing.
  - The 2D rFFT2 circulant convolution is expressed as dense DFT
    matmuls (cos/sin matrices, explicit real/imag arithmetic) so it all
    runs on the TensorEngine; the circulant embedding of the 111x111
    coefficient block into the 112x112 grid is folded into the DFT
    matrices as a phase shift (no scatter needed).

Everything runs in one pmap over the 8 cores via PJRT.
"""

import numpy as np
import jax
import jax.numpy as jnp
from functools import partial

EMBED_DIM = 512
NUM_HEADS = 8
D1 = 1024
HEAD_DIM = 128
RPE_DIM = 512
RPE_LAYERS = 3
N = 56
M = 56
BATCH = 8
EPS = 1e-8
P = 2 * N          # 112 padded FFT size (height)
Q = 2 * M          # 112 padded FFT size (width)
LW = Q // 2 + 1    # 57 rfft width bins
NCORES = 8
ROWS_PER_CORE = 14  # 8*14 = 112 >= 111 grid rows (last row masked out)

# ---------------------------------------------------------------- DFT mats
def _build_mats():
    n = np.arange(N)
    k = np.arange(P)
    l = np.arange(LW)
    # forward height DFT (56 -> 112 full complex), e^{-2pi i n k / P}
    ang_h = -2.0 * np.pi * np.outer(n, k) / P
    EHc = np.cos(ang_h).astype(np.float32)          # (56,112)
    EHs = np.sin(ang_h).astype(np.float32)
    # forward width rDFT (56 -> 57)
    ang_w = -2.0 * np.pi * np.outer(n, l) / Q
    EWc = np.cos(ang_w).astype(np.float32)          # (56,57)
    EWs = np.sin(ang_w).astype(np.float32)
    # coefficient DFTs with phase shift for circulant embedding:
    # row p of g corresponds to offset (p-55); position in circulant grid is
    # (p-55) mod 112, and e^{-2pi i k ((p-55) mod 112)/112} = e^{-2pi i k (p-55)/112}
    p_idx = np.arange(112)  # padded to 112 rows (row 111 is garbage, masked)
    ang_gh = -2.0 * np.pi * np.outer(p_idx - 55, k) / P
    GHc = np.cos(ang_gh).astype(np.float32)         # (112,112)
    GHs = np.sin(ang_gh).astype(np.float32)
    GHc[111] = 0.0  # mask padded garbage row
    GHs[111] = 0.0
    q_idx = np.arange(111)
    ang_gw = -2.0 * np.pi * np.outer(q_idx - 55, l) / Q
    GWc = np.cos(ang_gw).astype(np.float32)         # (111,57)
    GWs = np.sin(ang_gw).astype(np.float32)
    # inverse: y[n,m] = (1/P/Q) Re( sum_k sum_l w_l F[k,l] e^{2pi i(kn+lm)/112} )
    wl = np.ones(LW, np.float32) * 2.0
    wl[0] = 1.0
    wl[LW - 1] = 1.0
    ang_ih = 2.0 * np.pi * np.outer(k, n) / P
    IHc = np.cos(ang_ih).astype(np.float32)         # (112,56)
    IHs = np.sin(ang_ih).astype(np.float32)
    ang_iw = 2.0 * np.pi * np.outer(l, n) / Q
    IWc = (wl[:, None] * np.cos(ang_iw) / (P * Q)).astype(np.float32)  # (57,56)
    IWs = (wl[:, None] * np.sin(ang_iw) / (P * Q)).astype(np.float32)
    return EHc, EHs, EWc, EWs, GHc, GHs, GWc, GWs, IHc, IHs, IWc, IWs

_MATS = _build_mats()

# static relative-coordinate grid, padded to 112 rows
_DP = np.arange(-(N - 1), N + 1, dtype=np.float32)      # 112 values (last pad)
_DQ = np.arange(-(M - 1), M, dtype=np.float32)          # 111
_COORDS = np.stack(np.meshgrid(_DP, _DQ, indexing='ij'), axis=-1)  # (112,111,2)


def _rms(x):
    nrm = jnp.sqrt(jnp.sum(x * x, axis=-1, keepdims=True))
    return x / (nrm * (x.shape[-1] ** -0.5) + EPS)


def _core_fn(x_b, coords_sh, W_u, b_u, W_v, b_v, W_o, b_o,
             rpe_in_w, rpe_in_b, rpe_h_w, rpe_h_b, rpe_out_w, rpe_out_b):
    (EHc, EHs, EWc, EWs, GHc, GHs, GWc, GWs, IHc, IHs, IWc, IWs) = [
        jnp.asarray(m) for m in _MATS]

    # ---- RPE MLP on this core's shard of grid rows -> g shard
    t = coords_sh.reshape(-1, 2) @ rpe_in_w.T + rpe_in_b      # (14*111, 512)
    for i in range(RPE_LAYERS):
        t = jax.nn.relu(_rms(t)) @ rpe_h_w[i].T + rpe_h_b[i]
    g_sh = jax.nn.relu(_rms(t)) @ rpe_out_w.T + rpe_out_b     # (14*111, 1024)

    # ---- exchange: full coefficient tensor on every core
    g = jax.lax.all_gather(g_sh, 'i')                          # (8, 14*111, 1024)
    g = g.reshape(112, 111, D1)                                # row 111 = garbage (masked in GH)

    # ---- coefficient spectrum Af (112, 57, D1) complex
    # contract q first (111 -> 57, shrinking), then p (112 rows)
    t1r = jnp.einsum('pqc,ql->plc', g, GWc)
    t1i = jnp.einsum('pqc,ql->plc', g, GWs)
    Afr = jnp.einsum('plc,pk->klc', t1r, GHc) - jnp.einsum('plc,pk->klc', t1i, GHs)
    Afi = jnp.einsum('plc,pk->klc', t1r, GHs) + jnp.einsum('plc,pk->klc', t1i, GHc)

    # ---- main path for this core's batch element
    x = x_b                                                    # (56,56,512)
    xn = _rms(x)
    u = jax.nn.silu(xn @ W_u.T + b_u)                          # (56,56,1024)
    v = jax.nn.silu(xn @ W_v.T + b_v)                          # (56,56,1024)

    # forward 2D DFT of v (zero padding folded: only 56 rows/cols exist)
    # width rfft first (56 -> 57, shrinking), then height (56 -> 112)
    s1r = jnp.einsum('nmc,ml->nlc', v, EWc)                    # (56,57,D1)
    s1i = jnp.einsum('nmc,ml->nlc', v, EWs)
    Vfr = jnp.einsum('nlc,nk->klc', s1r, EHc) - jnp.einsum('nlc,nk->klc', s1i, EHs)
    Vfi = jnp.einsum('nlc,nk->klc', s1r, EHs) + jnp.einsum('nlc,nk->klc', s1i, EHc)

    # pointwise complex product
    Pr = Vfr * Afr - Vfi * Afi
    Pi = Vfr * Afi + Vfi * Afr

    # inverse: contract k then l, keep real part
    z1r = jnp.einsum('klc,kn->nlc', Pr, IHc) - jnp.einsum('klc,kn->nlc', Pi, IHs)
    z1i = jnp.einsum('klc,kn->nlc', Pr, IHs) + jnp.einsum('klc,kn->nlc', Pi, IHc)
    y = jnp.einsum('nlc,lm->nmc', z1r, IWc) - jnp.einsum('nlc,lm->nmc', z1i, IWs)

    out = (u * y) @ W_o.T + b_o + x                            # (56,56,512)
    return out


_PMAPPED = None


def _get_pmapped():
    global _PMAPPED
    if _PMAPPED is None:
        _PMAPPED = jax.pmap(
            _core_fn, axis_name='i',
            in_axes=(0, 0) + (None,) * 12,
            devices=jax.devices()[:NCORES])
    return _PMAPPED


def kernel(x, W_u, b_u, W_v, b_v, W_o, b_o,
           rpe_in_w, rpe_in_b, rpe_h_w, rpe_h_b, rpe_out_w, rpe_out_b,
           H, W):
    x = np.asarray(x, np.float32)
    coords = _COORDS.reshape(NCORES, ROWS_PER_CORE * 111, 2)
    f = _get_pmapped()
    out = f(x, jnp.asarray(coords),
            jnp.asarray(W_u), jnp.asarray(b_u),
            jnp.asarray(W_v), jnp.asarray(b_v),
            jnp.asarray(W_o), jnp.asarray(b_o),
            jnp.asarray(rpe_in_w), jnp.asarray(rpe_in_b),
            jnp.asarray(rpe_h_w), jnp.asarray(rpe_h_b),
            jnp.asarray(rpe_out_w), jnp.asarray(rpe_out_b))
    return np.asarray(out, np.float32)
